# revision 15
# baseline (speedup 1.0000x reference)
"""Trainium2 Bass kernel for quantized attention (qk int8 / pv fp8 path).

Shards the 16 (B,H) heads across 8 NeuronCores, 2 heads per core.
Per head: int8 block-quant-dequant of q (block 64) and mean-centered k
(block 128), fp8e4m3fn per-token quant-dequant of v, then
softmax(q_dq k_dq^T / sqrt(D)) @ v_dq in bf16.

Layout strategy: compute S^T tiles [k-seq partitions, q-seq free] so exp is a
single ACT pass from PSUM; PV uses v as the stationary operand producing O^T;
softmax denominators come from an extra ones-row matmul over p^T; final
PE-transposes give O in natural [seq, d] layout where the 1/denom scaling is a
native per-partition tensor_scalar.
"""

import math

import numpy as np

B, H, N, D = 2, 8, 2048, 128
NT = N // 128  # 16 seq tiles of 128
NC = 8  # cores
HPC = (B * H) // NC  # heads per core = 2
SM = 1.0 / math.sqrt(D)

_CACHE = {}


def _build_nc():
    import concourse.bass as bass  # noqa: F401
    import concourse.mybir as mybir
    import concourse.tile as tile
    from concourse import bacc, bass_isa
    from concourse.masks import make_identity

    f32 = mybir.dt.float32
    bf16 = mybir.dt.bfloat16
    i32 = mybir.dt.int32
    f8 = mybir.dt.float8e4
    AX = mybir.AxisListType
    OP = mybir.AluOpType
    AF = mybir.ActivationFunctionType

    nc = bacc.Bacc(None, target_bir_lowering=False)

    with tile.TileContext(nc) as tc:
        with (
            tc.tile_pool(name="dram", bufs=1, space="DRAM") as dram,
            tc.tile_pool(name="constp", bufs=1) as constp,
            tc.tile_pool(name="iop", bufs=2) as iop,
            tc.tile_pool(name="workp", bufs=1) as workp,
            tc.tile_pool(name="dqp", bufs=2) as dqp,
            tc.tile_pool(name="smallp", bufs=2) as smallp,
            tc.tile_pool(name="scrp", bufs=3) as scrp,
            tc.tile_pool(name="ps_b", bufs=3, space="PSUM") as ps_b,
            tc.tile_pool(name="ps_s", bufs=2, space="PSUM") as ps_s,
        ):
            q_d = dram.tile([HPC, N, D], f32, kind="ExternalInput", name="q", uniquify=False)
            k_d = dram.tile([HPC, N, D], f32, kind="ExternalInput", name="k", uniquify=False)
            v_d = dram.tile([HPC, N, D], f32, kind="ExternalInput", name="v", uniquify=False)
            o_d = dram.tile([HPC, N, D], bf16, kind="ExternalOutput", name="o", uniquify=False)

            ident = constp.tile([128, 128], f32)
            make_identity(nc, ident)
            ones_b = constp.tile([128, 1], bf16)
            nc.gpsimd.memset(ones_b[:], 1.0)

            for h in range(HPC):
                # ---- loads (natural layout [seq%128, seqtile, d]) ----
                q_nat = iop.tile([128, NT, 128], f32, tag="qnat")
                nc.sync.dma_start(out=q_nat[:], in_=q_d[h].rearrange("(t p) d -> p t d", p=128))
                k_nat = iop.tile([128, NT, 128], f32, tag="knat")
                nc.sync.dma_start(out=k_nat[:], in_=k_d[h].rearrange("(t p) d -> p t d", p=128))
                v_nat = iop.tile([128, NT, 128], f32, tag="vnat")
                nc.sync.dma_start(out=v_nat[:], in_=v_d[h].rearrange("(t p) d -> p t d", p=128))

                # ---- transpose raw q,k to [d, seq] via PE ----
                qT = workp.tile([128, N], f32, tag="qT")
                for g in range(4):  # 4 transposes per PSUM slot, one evac copy
                    tp = ps_s.tile([128, 512], f32, tag="t")
                    for u in range(4):
                        t = g * 4 + u
                        nc.tensor.transpose(tp[:, u * 128:(u + 1) * 128], q_nat[:, t, :], ident[:])
                    nc.vector.tensor_copy(qT[:, g * 512:(g + 1) * 512], tp[:])
                kT = workp.tile([128, N], f32, tag="kT")
                for g in range(4):
                    tp = ps_s.tile([128, 512], f32, tag="t")
                    for u in range(4):
                        t = g * 4 + u
                        nc.tensor.transpose(tp[:, u * 128:(u + 1) * 128], k_nat[:, t, :], ident[:])
                    nc.vector.tensor_copy(kT[:, g * 512:(g + 1) * 512], tp[:])

                # ---- center k along seq (free dim) ----
                ksum = smallp.tile([128, 1], f32, tag="ksum")
                nc.vector.reduce_sum(ksum[:], kT[:], axis=AX.X)
                kmean = smallp.tile([128, 1], f32, tag="kmean")
                nc.vector.tensor_scalar_mul(kmean[:], ksum[:], 1.0 / N)
                nc.gpsimd.tensor_scalar(
                    out=kT[:], in0=kT[:], scalar1=kmean[:], scalar2=None, op0=OP.subtract
                )

                # ---- block abs-max for q (block 64) and k (block 128) ----
                qbm = smallp.tile([128, 32], f32, tag="qbm")
                nc.vector.reduce_max(
                    qbm[:], qT[:].rearrange("p (b w) -> p b w", w=64), axis=AX.X,
                    apply_absolute_value=True,
                )
                kbm = smallp.tile([128, 16], f32, tag="kbm")
                nc.vector.reduce_max(
                    kbm[:], kT[:].rearrange("p (b w) -> p b w", w=128), axis=AX.X,
                    apply_absolute_value=True,
                )
                # ---- partition-dim max (gpsimd all-reduce broadcasts to all rows) ----
                qbma = smallp.tile([128, 32], f32, tag="qbma")
                nc.gpsimd.partition_all_reduce(qbma[:], qbm[:], 128, bass_isa.ReduceOp.max)
                kbma = smallp.tile([128, 16], f32, tag="kbma")
                nc.gpsimd.partition_all_reduce(kbma[:], kbm[:], 128, bass_isa.ReduceOp.max)

                # ---- scales + reciprocals (already in every partition) ----
                qs_t = smallp.tile([128, 32], f32, tag="qs")
                nc.vector.tensor_scalar_mul(qs_t[:], qbma[:], 1.0 / 127.0)
                qr_t = smallp.tile([128, 32], f32, tag="qr")
                nc.vector.reciprocal(qr_t[:], qs_t[:])
                ks_t = smallp.tile([128, 16], f32, tag="ks")
                nc.vector.tensor_scalar_mul(ks_t[:], kbma[:], 1.0 / 127.0)
                kr_t = smallp.tile([128, 16], f32, tag="kr")
                nc.vector.reciprocal(kr_t[:], ks_t[:])

                # ---- dequant q/k in T layout ----
                # HW fp32->int conversion rounds to nearest; emulate C-style
                # trunc: y = rne(t + 0.4995 - (t+0.4995 >= 0.4995)*0.999).
                DLT, GML = 0.4995, 0.999

                def dequant(src, nb, w, r_t, s_t, dq_out):
                    ti = workp.tile([128, N], f32, tag="tfull")
                    for b in range(nb):
                        sl = slice(b * w, (b + 1) * w)
                        nc.gpsimd.tensor_scalar(
                            out=ti[:, sl], in0=src[:, sl], scalar1=r_t[:, b:b + 1],
                            scalar2=DLT, op0=OP.mult, op1=OP.add,
                        )
                    shf = workp.tile([128, N], f32, tag="shift")
                    nc.gpsimd.tensor_scalar(
                        out=shf[:], in0=ti[:], scalar1=DLT, scalar2=GML,
                        op0=OP.is_ge, op1=OP.mult,
                    )
                    yi = workp.tile([128, N], i32, tag="qi")
                    nc.vector.tensor_tensor(out=yi[:], in0=ti[:], in1=shf[:], op=OP.subtract)
                    for b in range(nb):
                        sl = slice(b * w, (b + 1) * w)
                        nc.vector.tensor_scalar(
                            out=dq_out[:, sl], in0=yi[:, sl], scalar1=s_t[:, b:b + 1],
                            scalar2=None, op0=OP.mult,
                        )

                qdqT = dqp.tile([128, N], bf16, tag="qdq")
                dequant(qT, 32, 64, qr_t, qs_t, qdqT)
                kdqT = dqp.tile([128, N], bf16, tag="kdq")
                dequant(kT, 16, 128, kr_t, ks_t, kdqT)

                # ---- v fp8e4m3fn round-trip (448-grid = 4x the hw e4m3 240-grid) ----
                vam = smallp.tile([128, NT], f32, tag="vam")
                nc.vector.reduce_max(vam[:], v_nat[:], axis=AX.X, apply_absolute_value=True)
                vrc = smallp.tile([128, NT], f32, tag="vrc")
                nc.vector.reciprocal(vrc[:], vam[:])
                vdq = dqp.tile([128, NT, 128], bf16, tag="vdq")
                for t in range(NT):
                    t1 = scrp.tile([128, 128], f32, tag="t1")
                    nc.gpsimd.tensor_scalar(
                        out=t1[:], in0=v_nat[:, t, :], scalar1=vrc[:, t:t + 1],
                        scalar2=112.0, op0=OP.mult, op1=OP.mult,
                    )
                    v8 = scrp.tile([128, 128], f8, tag="v8")
                    nc.vector.tensor_copy(v8[:], t1[:])
                    nc.vector.tensor_scalar(
                        out=vdq[:, t, :], in0=v8[:], scalar1=vam[:, t:t + 1],
                        scalar2=1.0 / 112.0, op0=OP.mult, op1=OP.mult,
                    )

                # ---- attention ----
                o_sb = workp.tile([128, N], f32, tag="osb")
                den_sb = smallp.tile([1, N], f32, tag="densb")
                for ih in range(2):  # i-halves of 1024 queries
                    pT = workp.tile([128, NT, 1024], bf16, tag="pT")
                    for jt in range(NT):
                        sps = ps_b.tile([128, 1024], f32, tag="b")
                        for c in range(2):
                            nc.tensor.matmul(
                                out=sps[:, c * 512:(c + 1) * 512],
                                lhsT=kdqT[:, jt * 128:(jt + 1) * 128],
                                rhs=qdqT[:, ih * 1024 + c * 512: ih * 1024 + (c + 1) * 512],
                                start=True, stop=True,
                            )
                        nc.scalar.activation(out=pT[:, jt, :], in_=sps[:], func=AF.Exp, scale=SM)
                    for c2 in range(2):
                        osum = ps_b.tile([128, 512], f32, tag="b")
                        den = ps_s.tile([1, 512], f32, tag="t")
                        for jt in range(NT):
                            rhsp = pT[:, jt, c2 * 512:(c2 + 1) * 512]
                            nc.tensor.matmul(
                                out=osum[:], lhsT=vdq[:, jt, :], rhs=rhsp,
                                start=(jt == 0), stop=(jt == NT - 1),
                            )
                            nc.tensor.matmul(
                                out=den[:], lhsT=ones_b[:], rhs=rhsp,
                                start=(jt == 0), stop=(jt == NT - 1),
                            )
                        col = (ih * 2 + c2) * 512
                        nc.vector.tensor_copy(o_sb[:, col:col + 512], osum[:])
                        nc.vector.tensor_copy(den_sb[0:1, col:col + 512], den[:])

                # ---- denominators to [i%128, itile] layout, reciprocal ----
                den_col = smallp.tile([128, NT], f32, tag="dcol")
                dT = ps_s.tile([128, NT], f32, tag="t")
                for t in range(NT):
                    nc.tensor.transpose(dT[:, t:t + 1], den_sb[0:1, t * 128:(t + 1) * 128], ident[0:1, 0:1])
                nc.vector.tensor_copy(den_col[:], dT[:])
                rden = smallp.tile([128, NT], f32, tag="rden")
                nc.vector.reciprocal(rden[:], den_col[:])

                # ---- O^T -> O, normalize per-partition, store ----
                out_sb = iop.tile([128, NT, 128], bf16, tag="outsb")
                for t in range(NT):
                    oT = ps_s.tile([128, 128], f32, tag="t")
                    nc.tensor.transpose(oT[:], o_sb[:, t * 128:(t + 1) * 128], ident[:])
                    nc.vector.tensor_scalar(
                        out=out_sb[:, t, :], in0=oT[:], scalar1=rden[:, t:t + 1],
                        scalar2=None, op0=OP.mult,
                    )
                nc.sync.dma_start(out=o_d[h].rearrange("(t p) d -> p t d", p=128), in_=out_sb[:])

    nc.compile()
    return nc


def _get_nc():
    if "nc" not in _CACHE:
        _CACHE["nc"] = _build_nc()
    return _CACHE["nc"]


def kernel(q: np.ndarray, k: np.ndarray, v: np.ndarray, _trace: bool = False,
           _trace_kwargs=None):
    import ml_dtypes
    from concourse.bass_utils import run_bass_kernel_spmd

    nc = _get_nc()
    qf = np.ascontiguousarray(np.asarray(q, dtype=np.float32).reshape(B * H, N, D))
    kf = np.ascontiguousarray(np.asarray(k, dtype=np.float32).reshape(B * H, N, D))
    vf = np.ascontiguousarray(np.asarray(v, dtype=np.float32).reshape(B * H, N, D))

    in_maps = []
    for c in range(NC):
        sl = slice(c * HPC, (c + 1) * HPC)
        in_maps.append({
            "q": np.ascontiguousarray(qf[sl]),
            "k": np.ascontiguousarray(kf[sl]),
            "v": np.ascontiguousarray(vf[sl]),
        })

    kw = {}
    if _trace:
        kw = dict(trace=True, **(_trace_kwargs or {}))
    try:
        res = run_bass_kernel_spmd(nc, in_maps, core_ids=list(range(NC)), **kw)
    except Exception:
        # transient NRT_EXEC_UNIT_UNRECOVERABLE has been observed; retry once
        res = run_bass_kernel_spmd(nc, in_maps, core_ids=list(range(NC)), **kw)
    out = np.empty((B * H, N, D), dtype=ml_dtypes.bfloat16)
    for c in range(NC):
        out[c * HPC:(c + 1) * HPC] = np.asarray(res.results[c]["o"]).reshape(HPC, N, D)
    out = out.reshape(B, H, N, D)
    if _trace:
        return out, res
    return out


# revision 19
# speedup vs baseline: 1.0788x; 1.0788x over previous
"""Trainium2 Bass kernel for quantized attention (qk int8 / pv fp8 path).

Shards the 16 (B,H) heads across 8 NeuronCores, 2 heads per core.
Per head: int8 block-quant-dequant of q (block 64) and mean-centered k
(block 128), fp8e4m3fn per-token quant-dequant of v, then
softmax(q_dq k_dq^T / sqrt(D)) @ v_dq in bf16.

Layout strategy: compute S^T tiles [k-seq partitions, q-seq free] so exp is a
single ACT pass from PSUM; PV uses v as the stationary operand producing O^T;
softmax denominators come from an extra ones-row matmul over p^T; final
PE-transposes give O in natural [seq, d] layout where the 1/denom scaling is a
native per-partition tensor_scalar.
"""

import math

import numpy as np

B, H, N, D = 2, 8, 2048, 128
NT = N // 128  # 16 seq tiles of 128
NC = 8  # cores
HPC = (B * H) // NC  # heads per core = 2
SM = 1.0 / math.sqrt(D)

_CACHE = {}


def _build_nc():
    import concourse.bass as bass  # noqa: F401
    import concourse.mybir as mybir
    import concourse.tile as tile
    from concourse import bacc, bass_isa
    from concourse.masks import make_identity

    f32 = mybir.dt.float32
    bf16 = mybir.dt.bfloat16
    i32 = mybir.dt.int32
    f8 = mybir.dt.float8e4
    AX = mybir.AxisListType
    OP = mybir.AluOpType
    AF = mybir.ActivationFunctionType

    nc = bacc.Bacc(None, target_bir_lowering=False)

    with tile.TileContext(nc) as tc:
        with (
            tc.tile_pool(name="dram", bufs=1, space="DRAM") as dram,
            tc.tile_pool(name="constp", bufs=1) as constp,
            tc.tile_pool(name="iop", bufs=2) as iop,
            tc.tile_pool(name="workp", bufs=1) as workp,
            tc.tile_pool(name="dqp", bufs=2) as dqp,
            tc.tile_pool(name="smallp", bufs=2) as smallp,
            tc.tile_pool(name="scrp", bufs=3) as scrp,
            tc.tile_pool(name="ps_b", bufs=3, space="PSUM") as ps_b,
            tc.tile_pool(name="ps_s", bufs=2, space="PSUM") as ps_s,
        ):
            q_d = dram.tile([HPC, N, D], f32, kind="ExternalInput", name="q", uniquify=False)
            k_d = dram.tile([HPC, N, D], f32, kind="ExternalInput", name="k", uniquify=False)
            v_d = dram.tile([HPC, N, D], f32, kind="ExternalInput", name="v", uniquify=False)
            o_d = dram.tile([HPC, N, D], bf16, kind="ExternalOutput", name="o", uniquify=False)

            ident = constp.tile([128, 128], f32)
            make_identity(nc, ident)
            ones_b = constp.tile([128, 1], bf16)
            nc.gpsimd.memset(ones_b[:], 1.0)

            for h in range(HPC):
                # ---- loads (natural layout [seq%128, seqtile, d]) ----
                q_nat = iop.tile([128, NT, 128], f32, tag="qnat")
                nc.sync.dma_start(out=q_nat[:], in_=q_d[h].rearrange("(t p) d -> p t d", p=128))
                k_nat = iop.tile([128, NT, 128], f32, tag="knat")
                nc.sync.dma_start(out=k_nat[:], in_=k_d[h].rearrange("(t p) d -> p t d", p=128))
                v_nat = iop.tile([128, NT, 128], f32, tag="vnat", bufs=1)
                nc.sync.dma_start(out=v_nat[:], in_=v_d[h].rearrange("(t p) d -> p t d", p=128))

                # ---- transpose raw q,k to [d, seq] via PE ----
                qT = workp.tile([128, N], f32, tag="qT")
                for g in range(4):  # 4 transposes per PSUM slot, one evac copy
                    tp = ps_s.tile([128, 512], f32, tag="t")
                    for u in range(4):
                        t = g * 4 + u
                        nc.tensor.transpose(tp[:, u * 128:(u + 1) * 128], q_nat[:, t, :], ident[:])
                    nc.vector.tensor_copy(qT[:, g * 512:(g + 1) * 512], tp[:])
                kT = workp.tile([128, N], f32, tag="kT")
                for g in range(4):
                    tp = ps_s.tile([128, 512], f32, tag="t")
                    for u in range(4):
                        t = g * 4 + u
                        nc.tensor.transpose(tp[:, u * 128:(u + 1) * 128], k_nat[:, t, :], ident[:])
                    nc.vector.tensor_copy(kT[:, g * 512:(g + 1) * 512], tp[:])

                # ---- center k along seq (free dim) ----
                ksum = smallp.tile([128, 1], f32, tag="ksum")
                nc.vector.reduce_sum(ksum[:], kT[:], axis=AX.X)
                kmean = smallp.tile([128, 1], f32, tag="kmean")
                nc.vector.tensor_scalar_mul(kmean[:], ksum[:], 1.0 / N)
                nc.gpsimd.tensor_scalar(
                    out=kT[:], in0=kT[:], scalar1=kmean[:], scalar2=None, op0=OP.subtract
                )

                # ---- block abs-max for q (block 64) and k (block 128) ----
                qbm = smallp.tile([128, 32], f32, tag="qbm")
                nc.vector.reduce_max(
                    qbm[:], qT[:].rearrange("p (b w) -> p b w", w=64), axis=AX.X,
                    apply_absolute_value=True,
                )
                kbm = smallp.tile([128, 16], f32, tag="kbm")
                nc.vector.reduce_max(
                    kbm[:], kT[:].rearrange("p (b w) -> p b w", w=128), axis=AX.X,
                    apply_absolute_value=True,
                )
                # ---- partition-dim max (gpsimd all-reduce broadcasts to all rows) ----
                qbma = smallp.tile([128, 32], f32, tag="qbma")
                nc.gpsimd.partition_all_reduce(qbma[:], qbm[:], 128, bass_isa.ReduceOp.max)
                kbma = smallp.tile([128, 16], f32, tag="kbma")
                nc.gpsimd.partition_all_reduce(kbma[:], kbm[:], 128, bass_isa.ReduceOp.max)

                # ---- scales + reciprocals (already in every partition) ----
                qs_t = smallp.tile([128, 32], f32, tag="qs")
                nc.vector.tensor_scalar_mul(qs_t[:], qbma[:], 1.0 / 127.0)
                qr_t = smallp.tile([128, 32], f32, tag="qr")
                nc.vector.reciprocal(qr_t[:], qs_t[:])
                ks_t = smallp.tile([128, 16], f32, tag="ks")
                nc.vector.tensor_scalar_mul(ks_t[:], kbma[:], 1.0 / 127.0)
                kr_t = smallp.tile([128, 16], f32, tag="kr")
                nc.vector.reciprocal(kr_t[:], ks_t[:])

                # ---- dequant q/k in T layout ----
                # HW fp32->int conversion rounds to nearest; emulate C-style
                # trunc: y = rne(t + 0.4995 - (t+0.4995 >= 0.4995)*0.999).
                DLT, GML = 0.4995, 0.999

                def dequant(src, nb, w, r_t, s_t, dq_out):
                    ti = workp.tile([128, N], f32, tag="tfull")
                    for b in range(nb):
                        sl = slice(b * w, (b + 1) * w)
                        nc.gpsimd.tensor_scalar(
                            out=ti[:, sl], in0=src[:, sl], scalar1=r_t[:, b:b + 1],
                            scalar2=DLT, op0=OP.mult, op1=OP.add,
                        )
                    shf = workp.tile([128, N], f32, tag="shift")
                    nc.gpsimd.tensor_scalar(
                        out=shf[:], in0=ti[:], scalar1=DLT, scalar2=GML,
                        op0=OP.is_ge, op1=OP.mult,
                    )
                    yi = workp.tile([128, N], i32, tag="qi")
                    nc.vector.tensor_tensor(out=yi[:], in0=ti[:], in1=shf[:], op=OP.subtract)
                    for b in range(nb):
                        sl = slice(b * w, (b + 1) * w)
                        nc.vector.tensor_scalar(
                            out=dq_out[:, sl], in0=yi[:, sl], scalar1=s_t[:, b:b + 1],
                            scalar2=None, op0=OP.mult,
                        )

                qdqT = dqp.tile([128, N], bf16, tag="qdq")
                dequant(qT, 32, 64, qr_t, qs_t, qdqT)
                kdqT = dqp.tile([128, N], bf16, tag="kdq")
                dequant(kT, 16, 128, kr_t, ks_t, kdqT)

                # ---- v fp8e4m3fn round-trip (448-grid = 4x the hw e4m3 240-grid) ----
                vam = smallp.tile([128, NT], f32, tag="vam")
                nc.vector.reduce_max(vam[:], v_nat[:], axis=AX.X, apply_absolute_value=True)
                vrc = smallp.tile([128, NT], f32, tag="vrc")
                nc.vector.reciprocal(vrc[:], vam[:])
                vdq = dqp.tile([128, NT, 128], bf16, tag="vdq")
                for t in range(NT):
                    t1 = scrp.tile([128, 128], f32, tag="t1")
                    nc.gpsimd.tensor_scalar(
                        out=t1[:], in0=v_nat[:, t, :], scalar1=vrc[:, t:t + 1],
                        scalar2=112.0, op0=OP.mult, op1=OP.mult,
                    )
                    v8 = scrp.tile([128, 128], f8, tag="v8")
                    nc.vector.tensor_copy(v8[:], t1[:])
                    nc.vector.tensor_scalar(
                        out=vdq[:, t, :], in0=v8[:], scalar1=vam[:, t:t + 1],
                        scalar2=1.0 / 112.0, op0=OP.mult, op1=OP.mult,
                    )

                # ---- attention ----
                o_sb = workp.tile([128, N], f32, tag="osb")
                den_sb = smallp.tile([1, N], f32, tag="densb")
                for ih in range(2):  # i-halves of 1024 queries
                    pT = workp.tile([128, NT, 1024], bf16, tag="pT", bufs=2)
                    for jt in range(NT):
                        sps = ps_b.tile([128, 1024], f32, tag="b")
                        for c in range(2):
                            nc.tensor.matmul(
                                out=sps[:, c * 512:(c + 1) * 512],
                                lhsT=kdqT[:, jt * 128:(jt + 1) * 128],
                                rhs=qdqT[:, ih * 1024 + c * 512: ih * 1024 + (c + 1) * 512],
                                start=True, stop=True,
                            )
                        nc.scalar.activation(out=pT[:, jt, :], in_=sps[:], func=AF.Exp, scale=SM)
                    for c2 in range(2):
                        osum = ps_b.tile([128, 512], f32, tag="b")
                        den = ps_s.tile([1, 512], f32, tag="t")
                        for jt in range(NT):
                            rhsp = pT[:, jt, c2 * 512:(c2 + 1) * 512]
                            nc.tensor.matmul(
                                out=osum[:], lhsT=vdq[:, jt, :], rhs=rhsp,
                                start=(jt == 0), stop=(jt == NT - 1),
                            )
                            nc.tensor.matmul(
                                out=den[:], lhsT=ones_b[:], rhs=rhsp,
                                start=(jt == 0), stop=(jt == NT - 1),
                            )
                        col = (ih * 2 + c2) * 512
                        nc.scalar.copy(o_sb[:, col:col + 512], osum[:])
                        nc.scalar.copy(den_sb[0:1, col:col + 512], den[:])

                # ---- denominators to [i%128, itile] layout, reciprocal ----
                den_col = smallp.tile([128, NT], f32, tag="dcol")
                dT = ps_s.tile([128, NT], f32, tag="t")
                for t in range(NT):
                    nc.tensor.transpose(dT[:, t:t + 1], den_sb[0:1, t * 128:(t + 1) * 128], ident[0:1, 0:1])
                nc.vector.tensor_copy(den_col[:], dT[:])
                rden = smallp.tile([128, NT], f32, tag="rden")
                nc.vector.reciprocal(rden[:], den_col[:])

                # ---- O^T -> O, normalize per-partition, store ----
                out_sb = iop.tile([128, NT, 128], bf16, tag="outsb", bufs=1)
                for t in range(NT):
                    oT = ps_s.tile([128, 128], f32, tag="t")
                    nc.tensor.transpose(oT[:], o_sb[:, t * 128:(t + 1) * 128], ident[:])
                    nc.vector.tensor_scalar(
                        out=out_sb[:, t, :], in0=oT[:], scalar1=rden[:, t:t + 1],
                        scalar2=None, op0=OP.mult,
                    )
                nc.sync.dma_start(out=o_d[h].rearrange("(t p) d -> p t d", p=128), in_=out_sb[:])

    nc.compile()
    return nc


def _get_nc():
    if "nc" not in _CACHE:
        _CACHE["nc"] = _build_nc()
    return _CACHE["nc"]


def kernel(q: np.ndarray, k: np.ndarray, v: np.ndarray, _trace: bool = False,
           _trace_kwargs=None):
    import ml_dtypes
    from concourse.bass_utils import run_bass_kernel_spmd

    nc = _get_nc()
    qf = np.ascontiguousarray(np.asarray(q, dtype=np.float32).reshape(B * H, N, D))
    kf = np.ascontiguousarray(np.asarray(k, dtype=np.float32).reshape(B * H, N, D))
    vf = np.ascontiguousarray(np.asarray(v, dtype=np.float32).reshape(B * H, N, D))

    in_maps = []
    for c in range(NC):
        sl = slice(c * HPC, (c + 1) * HPC)
        in_maps.append({
            "q": np.ascontiguousarray(qf[sl]),
            "k": np.ascontiguousarray(kf[sl]),
            "v": np.ascontiguousarray(vf[sl]),
        })

    kw = {}
    if _trace:
        kw = dict(trace=True, **(_trace_kwargs or {}))
    try:
        res = run_bass_kernel_spmd(nc, in_maps, core_ids=list(range(NC)), **kw)
    except Exception:
        # transient NRT_EXEC_UNIT_UNRECOVERABLE has been observed; retry once
        res = run_bass_kernel_spmd(nc, in_maps, core_ids=list(range(NC)), **kw)
    out = np.empty((B * H, N, D), dtype=ml_dtypes.bfloat16)
    for c in range(NC):
        out[c * HPC:(c + 1) * HPC] = np.asarray(res.results[c]["o"]).reshape(HPC, N, D)
    out = out.reshape(B, H, N, D)
    if _trace:
        return out, res
    return out
